# revision 13
# baseline (speedup 1.0000x reference)
"""Trainium2 Bass kernel for the EnhancedNeuromorphicNetwork HH spiking net.

Strategy (pure batch data-parallel across 8 cores, B=512 -> 64 rows/core):
  - All HH state lives in SBUF, per-layer tiles [128 part, 512 free] in
    chunk-major layout: state[p, c*64 + b] for neuron j = c*128+p.
  - The two layers are pipelined: ScalarE computes the 6 transcendental
    rate activations for one layer while the DVE runs the gate/current/
    voltage update of the other layer, so both engines stay busy.
  - Three custom DVE ops fuse the hot math:
      HH_RATE : (v + K) * recipNR(1 - e)      (alpha_m / alpha_n rational)
      HH_M3H  : m^3 h
      HH_N4V  : n^4 * (v - e_k) * (DT g_k)
    plus scalar_tensor_tensor fusions for the gate updates
      g' = DT*a - (sigma - 1) * g,  sigma = DT*(a+b).
  - Layer-1 input current i1 = s0 @ (DT*W1) runs on the tensor engine with
    an extra K=1 "ones" row folding DT*b1 + leak-beta into the PSUM, so a
    single ScalarE copy materializes IEXT.
  - Spike accumulation acc += s1 is an identity-matmul into persistent PSUM.

The final output is (acc/T) @ w_out + b_out computed on-device, gathered
per-core as [OUT=128, 64] and reassembled on host.
"""
import math
from contextlib import ExitStack

import ml_dtypes
import numpy as np

import concourse.bacc as bacc
import concourse.bass as bass
import concourse.mybir as mybir
import concourse.tile as tile
from concourse.bass_utils import run_bass_kernel_spmd

DT = 0.1
B, IN, H0, H1, OUT = 512, 512, 1024, 1024, 128
E0 = int(0.8 * H0)
NCORES = 8
BC = B // NCORES          # batch per core (64)
KC0 = IN // 128           # K chunks for the input matmul (4)
NCH = H0 // 128           # H chunks (8)
FD = NCH * BC             # free dim per layer (512)

F32 = mybir.dt.float32
BF16 = mybir.dt.bfloat16
AF = mybir.ActivationFunctionType
ALU = mybir.AluOpType

# Chebyshev-minimax seed constants (same interval as RECIPROCAL_APPROX_FAST)
_RC0 = -0.23549792
_RC1 = 2.0017324
# sqrt(lambda) output-scale folds for the NR reciprocal (out = lambda/d)
_SQLM = math.sqrt(0.1 * DT)     # lambda_m = 0.1*DT  -> AMN_m = DT*a_m
_SQLN = math.sqrt(0.01 * DT)    # lambda_n = 0.01*DT -> AMN_n = DT*a_n


def _register_ops():
    """Register the three fused custom-DVE ops used by the HH update.

    HH_RATE_V : out = (in1 + imm2) * recip1NR(1 - in0)
                (one exponent-flip seed + one Newton-Raphson pass)
    HH_M3H    : out = in0^3 * in1
    HH_N4V    : out = in0^4 * (in1 - s0) * s1
    """
    from concourse import dve_ops as dvo
    from concourse.dve_spec import Spec, Src0, Src1, C0, C1, C2, One, Bin, AluOp
    from concourse.dve_spec import lower as dve_lower
    from concourse.dve_uop import DveOpSpec

    def reg(name, spec):
        for op in dvo.OPS:
            if op.name == name:
                return op
        shas = {}
        for ver in ("v3", "v4"):
            uops = dve_lower(spec, ver=ver)
            shas[ver] = DveOpSpec(name=name, opcode=0, uops=uops, rd1_en=True).sha(ver)
        op = dvo.DveOp(name, spec, subdim=False, uops_sha=shas)
        dvo.OPS.append(op)
        dvo.CUSTOM_DVE_SPECS[name] = spec
        dvo._SUB_OPCODE_FOR_NAME[name] = max(dvo._SUB_OPCODE_FOR_NAME.values()) + 1
        assert dvo._SUB_OPCODE_FOR_NAME[name] < 0x20
        return op

    def np_not(x):
        return (~np.asarray(x, np.float32).view(np.int32)).view(np.float32)

    # rate: d = 1 - in0 ; y0 = NOT(d)*s0 ; y1 = y0*(s1 - d*y0) ; out = in1*y1
    # (in1 carries the pre-scaled numerator, e.g. 0.01*(v+40))
    d = One - Src0
    nd = Bin(AluOp.BITWISE_NOT, d, d)
    y0 = nd * C0
    rate = Spec(
        body=Src1 * (y0 * (C1 - d * y0)),
        reference=lambda in0, in1, s0, s1, imm2: (
            (lambda dd, yy0: in1 * (yy0 * (s1 - dd * yy0)))(
                (1.0 - in0).astype(np.float32),
                np_not((1.0 - in0).astype(np.float32)) * np.float32(s0),
            )
        ),
    )
    m3h = Spec(
        body=((Src0 * Src0) * (Src0 * Src1)) * C0,
        reference=lambda in0, in1, s0, s1, imm2: (
            (in0 * in0) * (in0 * in1) * np.float32(s0)
        ),
    )
    sq_n = Src0 * Src0
    n4v = Spec(
        body=(sq_n * sq_n) * ((Src1 - C0) * C1),
        reference=lambda in0, in1, s0, s1, imm2: (
            (in0 * in0) * (in0 * in0) * ((in1 - np.float32(s0)) * np.float32(s1))
        ),
    )
    return reg("HH_RATE_V", rate), reg("HH_M3H", m3h), reg("HH_N4V", n4v)


def _build(T, scal, debug=False, b1_const=None):
    """Build the SPMD Bass module for `T` timesteps.

    scal: dict of python-float HH parameters (folded into immediates).
    """
    v_rest = scal["v_rest"]; v_th = scal["v_threshold"]; v_res = scal["v_reset"]
    gna = scal["g_na_max"]; gk = scal["g_k_max"]; gl = scal["g_leak"]
    ena = scal["e_na"]; ek = scal["e_k"]
    alpha = 1.0 - DT * gl          # leak folded into the v update
    # beta ( = DT*gl*v_rest ) is folded into the IEXT tiles host/bias side.
    ln = math.log

    op_rate, op_m3h, op_n4v = _register_ops()

    nc = bacc.Bacc()
    xT_d = nc.declare_dram_parameter("xT", [IN, BC], F32, isOutput=False)
    w0_d = nc.declare_dram_parameter("w_exc0", [IN, H0], F32, isOutput=False)
    b0_d = nc.declare_dram_parameter("b0dt", [128, NCH], F32, isOutput=False)
    w1_d = nc.declare_dram_parameter("w1dt", [H0, H1], BF16, isOutput=False)
    b1r_d = nc.declare_dram_parameter("b1row", [1, H1], BF16, isOutput=False)
    wo_d = nc.declare_dram_parameter("w_out", [H1, OUT], F32, isOutput=False)
    bo_d = nc.declare_dram_parameter("b_out", [128, 1], F32, isOutput=False)
    id_d = nc.declare_dram_parameter("ident", [128, 128], BF16, isOutput=False)
    out_d = nc.declare_dram_parameter("out", [OUT, BC], F32, isOutput=True)
    if debug:
        dbg_d = [nc.declare_dram_parameter(f"dbg{i}", [128, FD], F32, isOutput=True)
                 for i in range(8)]
        dbga_d = nc.declare_dram_parameter("dbg_acc", [128, FD], F32, isOutput=True)
    if debug == 2:
        dbgs_d = {n: nc.declare_dram_parameter(f"dbgs_{n}", [128, 2 * FD], F32,
                                               isOutput=True)
                  for n in ("E1", "E2", "AH", "TH", "AMN", "SMN", "UMN",
                            "P1", "P2", "P3", "ISUM", "SH1", "V1T", "IEXT")}

    with tile.TileContext(nc) as tc, ExitStack() as ctx:
        sb = ctx.enter_context(tc.tile_pool(name="sb", bufs=1))
        pp = ctx.enter_context(tc.tile_pool(name="pp", bufs=1, space="PSUM"))
        pi = ctx.enter_context(tc.tile_pool(name="pi", bufs=2, space="PSUM"))

        # ---- persistent SBUF tiles -----------------------------------
        w1sb = sb.tile([128, NCH * H1], BF16)        # DT*W1 chunk-major
        w0sb = sb.tile([128, KC0 * H0], F32)
        wosb = sb.tile([128, NCH * OUT], F32)
        xtsb = sb.tile([128, KC0 * BC], F32)
        b0sb = sb.tile([128, NCH], F32)
        b1row = sb.tile([1, H1], BF16)               # DT*b1 + beta
        bosb = sb.tile([128, 1], F32)
        idsb = sb.tile([128, 128], BF16)
        ones = sb.tile([1, BC], BF16)

        # per-layer state: gates [m|n|h] paired in one [128,1536] tile
        V = [sb.tile([128, FD], BF16, name=f"V{L}") for L in range(2)]
        G = [sb.tile([128, 3 * FD], BF16, name=f"G{L}") for L in range(2)]
        S = [sb.tile([128, FD], BF16, name=f"S{L}") for L in range(2)]
        # per-layer rate-activation outputs (ScalarE); A holds [DT*am|DT*an|DT*ah]
        E12 = [sb.tile([128, 2 * FD], F32, name=f"E12_{L}") for L in range(2)]
        VOFF = [sb.tile([128, 2 * FD], BF16, name=f"VOFF{L}") for L in range(2)]
        A = [sb.tile([128, 3 * FD], BF16, name=f"A{L}") for L in range(2)]
        BMN = [sb.tile([128, 2 * FD], BF16, name=f"BMN{L}") for L in range(2)]
        TH = [sb.tile([128, FD], BF16, name=f"TH{L}") for L in range(2)]
        TH2 = [sb.tile([128, FD], BF16, name=f"TH2_{L}") for L in range(2)]
        IEXT = [sb.tile([128, FD], BF16, name=f"IEXT{L}") for L in range(2)]
        V1T = [sb.tile([128, FD], BF16, name=f"V1T{L}") for L in range(2)]
        # shared scratch (written+consumed inside one DVE block)
        SMN = sb.tile([128, 2 * FD], BF16)
        SGA = sb.tile([128, 3 * FD], BF16)   # [sigma-1] for m,n,h
        UG = sb.tile([128, 3 * FD], BF16)
        P1 = sb.tile([128, FD], BF16)
        P2 = sb.tile([128, FD], BF16)
        P3 = sb.tile([128, FD], BF16)
        ISUM = sb.tile([128, FD], BF16)
        VRST = sb.tile([128, FD], BF16)
        RATE = sb.tile([128, FD], F32)
        OUTS = sb.tile([128, BC], F32)

        BIASC = sb.tile([128, 10], F32)      # activation bias constants

        accp = pp.tile([128, FD], F32)
        i0p = pp.tile([128, FD], F32)
        outp = pp.tile([128, BC], F32)

        # ---- loads (one DMA per tensor; chunk-major into partitions) --
        nc.sync.dma_start(w1sb[:].rearrange("p (c m) -> p c m", c=NCH),
                          w1_d[:].rearrange("(c p) m -> p c m", p=128))
        nc.sync.dma_start(w0sb[:].rearrange("p (c m) -> p c m", c=KC0),
                          w0_d[:].rearrange("(c p) m -> p c m", p=128))
        nc.sync.dma_start(xtsb[:].rearrange("p (c n) -> p c n", c=KC0),
                          xT_d[:].rearrange("(c p) n -> p c n", p=128))
        nc.sync.dma_start(wosb[:].rearrange("p (c o) -> p c o", c=NCH),
                          wo_d[:].rearrange("(c p) o -> p c o", p=128))
        nc.sync.dma_start(b0sb[:], b0_d[:])
        nc.sync.dma_start(b1row[:], b1r_d[:])
        nc.sync.dma_start(bosb[:], bo_d[:])
        nc.sync.dma_start(idsb[:], id_d[:])

        # ---- init -----------------------------------------------------
        for L in range(2):
            nc.vector.memset(V[L][:], v_rest)
            nc.vector.memset(G[L][:, :FD], 0.05)
            nc.vector.memset(G[L][:, FD:2 * FD], 0.32)
            nc.vector.memset(G[L][:, 2 * FD:], 0.6)
        nc.vector.memset(VRST[:], v_res)
        nc.gpsimd.memset(ones[:], 1.0)
        bias_vals = [-4.0,                                  # E1
                     -5.5,                                  # E2
                     -65.0 / 20.0 + ln(0.07 * DT),          # AH
                     -65.0 / 18.0 + ln(4.0 * DT),          # BM
                     -65.0 / 80.0 + ln(0.125 * DT),         # BN
                     35.0 / 20.0,                           # TH
                     DT / 2.0 - 1.0,                        # TH2
                     0.0 if b1_const is None else b1_const, # IEXT1 bias
                     0.4,                                   # VOFFm: .01*(v+40)
                     0.055]                                 # VOFFn: .001*(v+55)
        for i, bv in enumerate(bias_vals):
            nc.gpsimd.memset(BIASC[:, i:i + 1], bv)
        (bE1, bE2, bAH, bBM, bBN, bTH, bTH2, bB1, bVm, bVn) = (
            BIASC[:, i:i + 1] for i in range(10))

        # i0 = x_shard @ w_exc0 ;  IEXT[0] = (DT/T)*psum + DT*b0 + beta
        for m in range(NCH):
            for c in range(KC0):
                nc.tensor.matmul(
                    i0p[:, m * BC:(m + 1) * BC],
                    w0sb[:, c * H0 + m * 128: c * H0 + (m + 1) * 128],
                    xtsb[:, c * BC:(c + 1) * BC],
                    start=(c == 0), stop=(c == KC0 - 1))
        for m in range(NCH):
            nc.scalar.activation(IEXT[0][:, m * BC:(m + 1) * BC],
                                 i0p[:, m * BC:(m + 1) * BC],
                                 AF.Identity, bias=b0sb[:, m:m + 1],
                                 scale=DT / T)

        # ---- per-step building blocks --------------------------------
        def rates(L):
            """ScalarE: rate tiles for layer L (reads V[L])."""
            v = V[L][:]
            nc.scalar.activation(E12[L][:, :FD], v, AF.Exp, bias=bE1, scale=-0.1)
            nc.scalar.activation(E12[L][:, FD:], v, AF.Exp, bias=bE2, scale=-0.1)
            nc.scalar.activation(VOFF[L][:, :FD], v, AF.Identity, bias=bVm,
                                 scale=0.01)
            nc.scalar.activation(VOFF[L][:, FD:], v, AF.Identity, bias=bVn,
                                 scale=0.001)
            nc.scalar.activation(A[L][:, 2 * FD:], v, AF.Exp, bias=bAH,
                                 scale=-1.0 / 20.0)
            nc.scalar.activation(BMN[L][:, :FD], v, AF.Exp, bias=bBM,
                                 scale=-1.0 / 18.0)
            nc.scalar.activation(BMN[L][:, FD:], v, AF.Exp, bias=bBN,
                                 scale=-1.0 / 80.0)
            nc.scalar.activation(TH[L][:], v, AF.Tanh, bias=bTH, scale=1.0 / 20.0)
            nc.scalar.activation(TH2[L][:], TH[L][:], AF.Identity, bias=bTH2,
                                 scale=DT / 2.0)

        def dve_block(L):
            """DVE: full HH update for layer L (gates, currents, v, spike)."""
            v = V[L][:]
            g = G[L][:]
            stt = nc.vector.scalar_tensor_tensor
            tt = nc.vector.tensor_tensor
            # rational rates in one [1024] custom: A[:2FD] = [DT*a_m | DT*a_n]
            nc.vector._custom_dve(op_rate, out=A[L][:, :2 * FD],
                                  in0=E12[L][:], in1=VOFF[L][:],
                                  s0=_RC0, s1=_RC1, imm2=0.0)
            # sigma-1 for all gates -> SGA [m|n|h]
            tt(SMN[:], A[L][:, :2 * FD], BMN[L][:], ALU.add)
            nc.vector.tensor_scalar(SGA[:, :2 * FD], SMN[:], 1.0, None,
                                    ALU.subtract)
            tt(SGA[:, 2 * FD:], TH2[L][:], A[L][:, 2 * FD:], ALU.add)
            # fused 3-gate update: g' = DT*a - (sigma-1)*g
            tt(UG[:], SGA[:], g, ALU.mult)
            tt(g, A[L][:], UG[:], ALU.subtract)
            # currents: P1 = DT*gna*m^3 h ; P2 = DT*gk*n^4*(v-ek)
            nc.vector._custom_dve(op_m3h, out=P1[:], in0=G[L][:, :FD],
                                  in1=G[L][:, 2 * FD:], s0=DT * gna, s1=0.0,
                                  imm2=0.0)
            nc.vector._custom_dve(op_n4v, out=P2[:], in0=G[L][:, FD:2 * FD],
                                  in1=v, s0=ek, s1=DT * gk, imm2=0.0)
            stt(P3[:], v, ena, P1[:], ALU.subtract, ALU.mult)
            tt(ISUM[:], P3[:], P2[:], ALU.add)
            # v' = (alpha*v - ISUM) + IEXT ; only the last (short) op
            # depends on IEXT, which for layer 1 arrives late via PE+copy
            stt(V1T[L][:], v, alpha, ISUM[:], ALU.mult, ALU.subtract)
            tt(v, V1T[L][:], IEXT[L][:], ALU.add)
            # spike + reset (bf16 spike tile doubles as the int mask)
            nc.vector.tensor_scalar(S[L][:], v, v_th, None, ALU.is_gt)
            nc.vector.copy_predicated(v, S[L][:].bitcast(mybir.dt.uint16),
                                      VRST[:])

        def pe_i1(k):
            """i1 = s0 @ (DT*W1) + (DT*b1 + beta) into fresh PSUM tile."""
            i1p = pi.tile([128, FD], F32, tag="i1p")
            for m in range(NCH):
                for c in range(NCH):
                    nc.tensor.matmul(
                        i1p[:, m * BC:(m + 1) * BC],
                        w1sb[:, c * H1 + m * 128: c * H1 + (m + 1) * 128],
                        S[0][:, c * BC:(c + 1) * BC],
                        start=(c == 0),
                        stop=(b1_const is not None and c == NCH - 1))
                if b1_const is None:
                    nc.tensor.matmul(
                        i1p[:, m * BC:(m + 1) * BC],
                        b1row[0:1, m * 128:(m + 1) * 128],
                        ones[0:1, :],
                        start=False, stop=True)
            return i1p

        # ---- the pipelined timestep loop -----------------------------
        rates(0)
        for k in range(T):
            rates(1)                       # ScalarE (runs during DVE layer-0)
            dve_block(0)                   # DVE layer 0 step k
            if debug == 2 and k == 0:
                srcs = dict(SMN=SMN, P1=P1, P2=P2, P3=P3,
                            ISUM=ISUM, V1T=V1T[0], IEXT=IEXT[0])
                for n, t in srcs.items():
                    fd = t.shape[-1]
                    DBGS = sb.tile([128, fd], F32, name=f"DBGS_{n}")
                    nc.vector.tensor_copy(DBGS[:], t[:])
                    nc.sync.dma_start(dbgs_d[n][:, :fd], DBGS[:])
            i1p = pe_i1(k)                 # PE
            if b1_const is None:
                nc.scalar.copy(IEXT[1][:], i1p[:])   # ScalarE PSUM -> SBUF
            else:
                nc.scalar.activation(IEXT[1][:], i1p[:], AF.Identity,
                                     bias=bB1, scale=1.0)
            if k + 1 < T:
                rates(0)                   # ScalarE (runs during DVE layer-1)
            dve_block(1)                   # DVE layer 1 step k
            nc.tensor.matmul(accp[:], idsb[:], S[1][:],
                             start=(k == 0), stop=(k == T - 1),
                             skip_group_check=True)

        # ---- readout: (acc/T) @ w_out + b_out ------------------------
        nc.scalar.activation(RATE[:], accp[:], AF.Identity, bias=0.0, scale=1.0 / T)
        for c in range(NCH):
            nc.tensor.matmul(outp[:],
                             wosb[:, c * OUT:(c + 1) * OUT],
                             RATE[:, c * BC:(c + 1) * BC],
                             start=(c == 0), stop=(c == NCH - 1))
        nc.scalar.activation(OUTS[:], outp[:], AF.Identity, bias=bosb[:, 0:1], scale=1.0)
        nc.sync.dma_start(out_d[:], OUTS[:])

        if debug:
            dbg_src = [V[0][:], G[0][:, :FD], G[0][:, 2 * FD:], G[0][:, FD:2 * FD],
                       V[1][:], G[1][:, :FD], G[1][:, 2 * FD:], G[1][:, FD:2 * FD]]
            for i, sap in enumerate(dbg_src):
                DBG = sb.tile([128, FD], F32, name=f"DBG{i}")
                nc.vector.tensor_copy(DBG[:], sap)
                nc.sync.dma_start(dbg_d[i][:], DBG[:])
            nc.scalar.activation(RATE[:], accp[:], AF.Identity, bias=0.0, scale=1.0)
            nc.sync.dma_start(dbga_d[:], RATE[:])
    nc.compile()
    return nc


_NC_CACHE = {}


def _get_nc(T, scal, debug=False, b1_const=None):
    key = (T, tuple(sorted(scal.items())), debug, b1_const)
    if key not in _NC_CACHE:
        _NC_CACHE[key] = _build(T, scal, debug, b1_const=b1_const)
    return _NC_CACHE[key]


def _chunk_major(vec):
    """[1024] -> [128, 8] with vec[c*128+p] at [p, c]."""
    return np.ascontiguousarray(vec.reshape(NCH, 128).T)


def _make_in_maps(inputs, T, scal):
    gl = scal["g_leak"]; v_rest = scal["v_rest"]
    beta = DT * gl * v_rest

    x = np.asarray(inputs["x"], np.float32)
    w_exc0 = np.ascontiguousarray(np.asarray(inputs["w_exc0"], np.float32))
    W1 = np.concatenate([np.asarray(inputs["w_exc1"], np.float32),
                         -np.asarray(inputs["w_inh1"], np.float32)], axis=0)
    w1dt = (DT * W1).astype(ml_dtypes.bfloat16)
    b0dt = (_chunk_major(DT * np.asarray(inputs["b_exc0"], np.float32)) + beta
            ).astype(np.float32)
    b1row = (DT * (np.asarray(inputs["b_exc1"], np.float32)
                   - np.asarray(inputs["b_inh1"], np.float32)) + beta
             ).reshape(1, H1).astype(ml_dtypes.bfloat16)
    w_out = np.ascontiguousarray(np.asarray(inputs["w_out"], np.float32))
    b_out = np.asarray(inputs["b_out"], np.float32).reshape(128, 1)
    ident = np.eye(128, dtype=ml_dtypes.bfloat16)

    in_maps = []
    for c in range(NCORES):
        xT = np.ascontiguousarray(x[c * BC:(c + 1) * BC, :].T)
        in_maps.append({
            "xT": xT, "w_exc0": w_exc0, "b0dt": b0dt, "w1dt": w1dt,
            "b1row": b1row, "w_out": w_out, "b_out": b_out, "ident": ident,
        })
    return in_maps


def _b1_const(inputs, scal):
    """If b_exc1 - b_inh1 is a uniform constant, the PE bias rows can be
    dropped and DT*b1 + beta folded into the IEXT copy bias."""
    b1 = (np.asarray(inputs["b_exc1"], np.float32)
          - np.asarray(inputs["b_inh1"], np.float32))
    if np.all(b1 == b1.flat[0]):
        beta = DT * scal["g_leak"] * scal["v_rest"]
        return float(DT * b1.flat[0] + beta)
    return None


def kernel(**inputs):
    T = int(np.asarray(inputs["timesteps"]))
    scal = {k: float(np.asarray(inputs[k])) for k in
            ("v_rest", "v_threshold", "v_reset", "g_na_max", "g_k_max",
             "g_leak", "e_na", "e_k")}
    nc = _get_nc(T, scal, b1_const=_b1_const(inputs, scal))
    in_maps = _make_in_maps(inputs, T, scal)
    res = run_bass_kernel_spmd(nc, in_maps, core_ids=list(range(NCORES)))
    out = np.empty((B, OUT), np.float32)
    for c in range(NCORES):
        out[c * BC:(c + 1) * BC, :] = res.results[c]["out"].T
    return out


# revision 14
# speedup vs baseline: 1.0514x; 1.0514x over previous
"""Trainium2 Bass kernel for the EnhancedNeuromorphicNetwork HH spiking net.

Strategy (pure batch data-parallel across 8 cores, B=512 -> 64 rows/core):
  - All HH state lives in SBUF, per-layer tiles [128 part, 512 free] in
    chunk-major layout: state[p, c*64 + b] for neuron j = c*128+p.
  - The two layers are pipelined: ScalarE computes the 6 transcendental
    rate activations for one layer while the DVE runs the gate/current/
    voltage update of the other layer, so both engines stay busy.
  - Three custom DVE ops fuse the hot math:
      HH_RATE : (v + K) * recipNR(1 - e)      (alpha_m / alpha_n rational)
      HH_M3H  : m^3 h
      HH_N4V  : n^4 * (v - e_k) * (DT g_k)
    plus scalar_tensor_tensor fusions for the gate updates
      g' = DT*a - (sigma - 1) * g,  sigma = DT*(a+b).
  - Layer-1 input current i1 = s0 @ (DT*W1) runs on the tensor engine with
    an extra K=1 "ones" row folding DT*b1 + leak-beta into the PSUM, so a
    single ScalarE copy materializes IEXT.
  - Spike accumulation acc += s1 is an identity-matmul into persistent PSUM.

The final output is (acc/T) @ w_out + b_out computed on-device, gathered
per-core as [OUT=128, 64] and reassembled on host.
"""
import math
from contextlib import ExitStack

import ml_dtypes
import numpy as np

import concourse.bacc as bacc
import concourse.bass as bass
import concourse.mybir as mybir
import concourse.tile as tile
from concourse.bass_utils import run_bass_kernel_spmd

DT = 0.1
B, IN, H0, H1, OUT = 512, 512, 1024, 1024, 128
E0 = int(0.8 * H0)
NCORES = 8
BC = B // NCORES          # batch per core (64)
KC0 = IN // 128           # K chunks for the input matmul (4)
NCH = H0 // 128           # H chunks (8)
FD = NCH * BC             # free dim per layer (512)

F32 = mybir.dt.float32
BF16 = mybir.dt.bfloat16
AF = mybir.ActivationFunctionType
ALU = mybir.AluOpType

# Chebyshev-minimax seed constants (same interval as RECIPROCAL_APPROX_FAST)
_RC0 = -0.23549792
_RC1 = 2.0017324
# sqrt(lambda) output-scale folds for the NR reciprocal (out = lambda/d)
_SQLM = math.sqrt(0.1 * DT)     # lambda_m = 0.1*DT  -> AMN_m = DT*a_m
_SQLN = math.sqrt(0.01 * DT)    # lambda_n = 0.01*DT -> AMN_n = DT*a_n


def _register_ops():
    """Register the three fused custom-DVE ops used by the HH update.

    HH_RATE_V : out = (in1 + imm2) * recip1NR(1 - in0)
                (one exponent-flip seed + one Newton-Raphson pass)
    HH_M3H    : out = in0^3 * in1
    HH_N4V    : out = in0^4 * (in1 - s0) * s1
    """
    from concourse import dve_ops as dvo
    from concourse.dve_spec import Spec, Src0, Src1, C0, C1, C2, One, Bin, AluOp
    from concourse.dve_spec import lower as dve_lower
    from concourse.dve_uop import DveOpSpec

    def reg(name, spec):
        for op in dvo.OPS:
            if op.name == name:
                return op
        shas = {}
        for ver in ("v3", "v4"):
            uops = dve_lower(spec, ver=ver)
            shas[ver] = DveOpSpec(name=name, opcode=0, uops=uops, rd1_en=True).sha(ver)
        op = dvo.DveOp(name, spec, subdim=False, uops_sha=shas)
        dvo.OPS.append(op)
        dvo.CUSTOM_DVE_SPECS[name] = spec
        dvo._SUB_OPCODE_FOR_NAME[name] = max(dvo._SUB_OPCODE_FOR_NAME.values()) + 1
        assert dvo._SUB_OPCODE_FOR_NAME[name] < 0x20
        return op

    def np_not(x):
        return (~np.asarray(x, np.float32).view(np.int32)).view(np.float32)

    # rate: d = 1 - in0 ; y0 = NOT(d)*s0 ; y1 = y0*(s1 - d*y0) ; out = in1*y1
    # (in1 carries the pre-scaled numerator, e.g. 0.01*(v+40))
    d = One - Src0
    nd = Bin(AluOp.BITWISE_NOT, d, d)
    y0 = nd * C0
    rate = Spec(
        body=Src1 * (y0 * (C1 - d * y0)),
        reference=lambda in0, in1, s0, s1, imm2: (
            (lambda dd, yy0: in1 * (yy0 * (s1 - dd * yy0)))(
                (1.0 - in0).astype(np.float32),
                np_not((1.0 - in0).astype(np.float32)) * np.float32(s0),
            )
        ),
    )
    m3h = Spec(
        body=((Src0 * Src0) * (Src0 * Src1)) * C0,
        reference=lambda in0, in1, s0, s1, imm2: (
            (in0 * in0) * (in0 * in1) * np.float32(s0)
        ),
    )
    sq_n = Src0 * Src0
    n4v = Spec(
        body=(sq_n * sq_n) * ((Src1 - C0) * C1),
        reference=lambda in0, in1, s0, s1, imm2: (
            (in0 * in0) * (in0 * in0) * ((in1 - np.float32(s0)) * np.float32(s1))
        ),
    )
    return reg("HH_RATE_V", rate), reg("HH_M3H", m3h), reg("HH_N4V", n4v)


def _build(T, scal, debug=False, b1_const=None):
    """Build the SPMD Bass module for `T` timesteps.

    scal: dict of python-float HH parameters (folded into immediates).
    """
    v_rest = scal["v_rest"]; v_th = scal["v_threshold"]; v_res = scal["v_reset"]
    gna = scal["g_na_max"]; gk = scal["g_k_max"]; gl = scal["g_leak"]
    ena = scal["e_na"]; ek = scal["e_k"]
    alpha = 1.0 - DT * gl          # leak folded into the v update
    # beta ( = DT*gl*v_rest ) is folded into the IEXT tiles host/bias side.
    ln = math.log

    op_rate, op_m3h, op_n4v = _register_ops()

    nc = bacc.Bacc()
    xT_d = nc.declare_dram_parameter("xT", [IN, BC], F32, isOutput=False)
    w0_d = nc.declare_dram_parameter("w_exc0", [IN, H0], F32, isOutput=False)
    b0_d = nc.declare_dram_parameter("b0dt", [128, NCH], F32, isOutput=False)
    w1_d = nc.declare_dram_parameter("w1dt", [H0, H1], BF16, isOutput=False)
    b1r_d = nc.declare_dram_parameter("b1row", [1, H1], BF16, isOutput=False)
    wo_d = nc.declare_dram_parameter("w_out", [H1, OUT], F32, isOutput=False)
    bo_d = nc.declare_dram_parameter("b_out", [128, 1], F32, isOutput=False)
    id_d = nc.declare_dram_parameter("ident", [128, 128], BF16, isOutput=False)
    out_d = nc.declare_dram_parameter("out", [OUT, BC], F32, isOutput=True)
    if debug:
        dbg_d = [nc.declare_dram_parameter(f"dbg{i}", [128, FD], F32, isOutput=True)
                 for i in range(8)]
        dbga_d = nc.declare_dram_parameter("dbg_acc", [128, FD], F32, isOutput=True)
    if debug == 2:
        dbgs_d = {n: nc.declare_dram_parameter(f"dbgs_{n}", [128, 2 * FD], F32,
                                               isOutput=True)
                  for n in ("E1", "E2", "AH", "TH", "AMN", "SMN", "UMN",
                            "P1", "P2", "P3", "ISUM", "SH1", "V1T", "IEXT")}

    with tile.TileContext(nc) as tc, ExitStack() as ctx:
        sb = ctx.enter_context(tc.tile_pool(name="sb", bufs=1))
        pp = ctx.enter_context(tc.tile_pool(name="pp", bufs=1, space="PSUM"))
        pi = ctx.enter_context(tc.tile_pool(name="pi", bufs=2, space="PSUM"))

        # ---- persistent SBUF tiles -----------------------------------
        w1sb = sb.tile([128, NCH * H1], BF16)        # DT*W1 chunk-major
        w0sb = sb.tile([128, KC0 * H0], F32)
        wosb = sb.tile([128, NCH * OUT], F32)
        xtsb = sb.tile([128, KC0 * BC], F32)
        b0sb = sb.tile([128, NCH], F32)
        b1row = sb.tile([1, H1], BF16)               # DT*b1 + beta
        bosb = sb.tile([128, 1], F32)
        idsb = sb.tile([128, 128], BF16)
        ones = sb.tile([1, BC], BF16)

        # per-layer state: gates [m|n|h] paired in one [128,1536] tile
        V = [sb.tile([128, FD], BF16, name=f"V{L}") for L in range(2)]
        G = [sb.tile([128, 3 * FD], BF16, name=f"G{L}") for L in range(2)]
        S = [sb.tile([128, FD], BF16, name=f"S{L}") for L in range(2)]
        # per-layer rate-activation outputs (ScalarE); A holds [DT*am|DT*an|DT*ah]
        E12 = [sb.tile([128, 2 * FD], F32, name=f"E12_{L}") for L in range(2)]
        VOFF = [sb.tile([128, 2 * FD], BF16, name=f"VOFF{L}") for L in range(2)]
        A = [sb.tile([128, 3 * FD], BF16, name=f"A{L}") for L in range(2)]
        BMN = [sb.tile([128, 2 * FD], BF16, name=f"BMN{L}") for L in range(2)]
        TH = [sb.tile([128, FD], BF16, name=f"TH{L}") for L in range(2)]
        TH2 = [sb.tile([128, FD], BF16, name=f"TH2_{L}") for L in range(2)]
        IEXT = [sb.tile([128, FD], BF16, name=f"IEXT{L}") for L in range(2)]
        V1T = [sb.tile([128, FD], BF16, name=f"V1T{L}") for L in range(2)]
        # shared scratch (written+consumed inside one DVE block)
        SMN = sb.tile([128, 2 * FD], BF16)
        SGA = sb.tile([128, 3 * FD], BF16)   # [sigma-1] for m,n,h
        UG = sb.tile([128, 3 * FD], BF16)
        P1 = sb.tile([128, FD], BF16)
        P2 = sb.tile([128, FD], BF16)
        P3 = sb.tile([128, FD], BF16)
        ISUM = sb.tile([128, FD], BF16)
        VRST = sb.tile([128, FD], BF16)
        RATE = sb.tile([128, FD], F32)
        OUTS = sb.tile([128, BC], F32)

        BIASC = sb.tile([128, 10], F32)      # activation bias constants

        accp = pp.tile([128, FD], F32)
        i0p = pp.tile([128, FD], F32)
        outp = pp.tile([128, BC], F32)

        # ---- loads (one DMA per tensor; chunk-major into partitions) --
        nc.sync.dma_start(w1sb[:].rearrange("p (c m) -> p c m", c=NCH),
                          w1_d[:].rearrange("(c p) m -> p c m", p=128))
        nc.sync.dma_start(w0sb[:].rearrange("p (c m) -> p c m", c=KC0),
                          w0_d[:].rearrange("(c p) m -> p c m", p=128))
        nc.sync.dma_start(xtsb[:].rearrange("p (c n) -> p c n", c=KC0),
                          xT_d[:].rearrange("(c p) n -> p c n", p=128))
        nc.sync.dma_start(wosb[:].rearrange("p (c o) -> p c o", c=NCH),
                          wo_d[:].rearrange("(c p) o -> p c o", p=128))
        nc.sync.dma_start(b0sb[:], b0_d[:])
        nc.sync.dma_start(b1row[:], b1r_d[:])
        nc.sync.dma_start(bosb[:], bo_d[:])
        nc.sync.dma_start(idsb[:], id_d[:])

        # ---- init -----------------------------------------------------
        for L in range(2):
            nc.vector.memset(V[L][:], v_rest)
            nc.vector.memset(G[L][:, :FD], 0.05)
            nc.vector.memset(G[L][:, FD:2 * FD], 0.32)
            nc.vector.memset(G[L][:, 2 * FD:], 0.6)
        nc.vector.memset(VRST[:], v_res)
        nc.gpsimd.memset(ones[:], 1.0)
        bias_vals = [-4.0,                                  # E1
                     -5.5,                                  # E2
                     -65.0 / 20.0 + ln(0.07 * DT),          # AH
                     -65.0 / 18.0 + ln(4.0 * DT),          # BM
                     -65.0 / 80.0 + ln(0.125 * DT),         # BN
                     35.0 / 20.0,                           # TH
                     DT / 2.0 - 1.0,                        # TH2
                     0.0 if b1_const is None else b1_const, # IEXT1 bias
                     0.4,                                   # VOFFm: .01*(v+40)
                     0.055]                                 # VOFFn: .001*(v+55)
        for i, bv in enumerate(bias_vals):
            nc.gpsimd.memset(BIASC[:, i:i + 1], bv)
        (bE1, bE2, bAH, bBM, bBN, bTH, bTH2, bB1, bVm, bVn) = (
            BIASC[:, i:i + 1] for i in range(10))

        # i0 = x_shard @ w_exc0 ;  IEXT[0] = (DT/T)*psum + DT*b0 + beta
        for m in range(NCH):
            for c in range(KC0):
                nc.tensor.matmul(
                    i0p[:, m * BC:(m + 1) * BC],
                    w0sb[:, c * H0 + m * 128: c * H0 + (m + 1) * 128],
                    xtsb[:, c * BC:(c + 1) * BC],
                    start=(c == 0), stop=(c == KC0 - 1))
        for m in range(NCH):
            nc.scalar.activation(IEXT[0][:, m * BC:(m + 1) * BC],
                                 i0p[:, m * BC:(m + 1) * BC],
                                 AF.Identity, bias=b0sb[:, m:m + 1],
                                 scale=DT / T)

        # ---- per-step building blocks --------------------------------
        def rates(L):
            """ScalarE: rate tiles for layer L (reads V[L])."""
            v = V[L][:]
            nc.scalar.activation(E12[L][:, :FD], v, AF.Exp, bias=bE1, scale=-0.1)
            nc.scalar.activation(E12[L][:, FD:], v, AF.Exp, bias=bE2, scale=-0.1)
            nc.scalar.activation(VOFF[L][:, :FD], v, AF.Identity, bias=bVm,
                                 scale=0.01)
            nc.scalar.activation(VOFF[L][:, FD:], v, AF.Identity, bias=bVn,
                                 scale=0.001)
            nc.scalar.activation(BMN[L][:, :FD], v, AF.Exp, bias=bBM,
                                 scale=-1.0 / 18.0)
            nc.scalar.activation(BMN[L][:, FD:], v, AF.Exp, bias=bBN,
                                 scale=-1.0 / 80.0)
            nc.scalar.activation(TH[L][:], v, AF.Tanh, bias=bTH, scale=1.0 / 20.0)
            nc.scalar.activation(TH2[L][:], TH[L][:], AF.Identity, bias=bTH2,
                                 scale=DT / 2.0)
            nc.scalar.activation(A[L][:, 2 * FD:], v, AF.Exp, bias=bAH,
                                 scale=-1.0 / 20.0)

        def dve_block(L, iext=None):
            """DVE: full HH update for layer L (gates, currents, v, spike)."""
            v = V[L][:]
            iext = IEXT[L][:] if iext is None else iext
            g = G[L][:]
            stt = nc.vector.scalar_tensor_tensor
            tt = nc.vector.tensor_tensor
            # rational rates in one [1024] custom: A[:2FD] = [DT*a_m | DT*a_n]
            nc.vector._custom_dve(op_rate, out=A[L][:, :2 * FD],
                                  in0=E12[L][:], in1=VOFF[L][:],
                                  s0=_RC0, s1=_RC1, imm2=0.0)
            # sigma-1 for all gates -> SGA [m|n|h]
            tt(SMN[:], A[L][:, :2 * FD], BMN[L][:], ALU.add)
            nc.vector.tensor_scalar(SGA[:, :2 * FD], SMN[:], 1.0, None,
                                    ALU.subtract)
            tt(SGA[:, 2 * FD:], TH2[L][:], A[L][:, 2 * FD:], ALU.add)
            # fused 3-gate update: g' = DT*a - (sigma-1)*g
            tt(UG[:], SGA[:], g, ALU.mult)
            tt(g, A[L][:], UG[:], ALU.subtract)
            # currents: P1 = DT*gna*m^3 h ; P2 = DT*gk*n^4*(v-ek)
            nc.vector._custom_dve(op_m3h, out=P1[:], in0=G[L][:, :FD],
                                  in1=G[L][:, 2 * FD:], s0=DT * gna, s1=0.0,
                                  imm2=0.0)
            nc.vector._custom_dve(op_n4v, out=P2[:], in0=G[L][:, FD:2 * FD],
                                  in1=v, s0=ek, s1=DT * gk, imm2=0.0)
            stt(P3[:], v, ena, P1[:], ALU.subtract, ALU.mult)
            tt(ISUM[:], P3[:], P2[:], ALU.add)
            # v' = (alpha*v - ISUM) + IEXT ; only the last (short) op
            # depends on IEXT, which for layer 1 arrives late via PE+copy
            stt(V1T[L][:], v, alpha, ISUM[:], ALU.mult, ALU.subtract)
            tt(v, V1T[L][:], iext, ALU.add)
            # spike + reset (bf16 spike tile doubles as the int mask)
            nc.vector.tensor_scalar(S[L][:], v, v_th, None, ALU.is_gt)
            nc.vector.copy_predicated(v, S[L][:].bitcast(mybir.dt.uint16),
                                      VRST[:])

        def pe_i1(k):
            """i1 = s0 @ (DT*W1) + (DT*b1 + beta) into fresh PSUM tile."""
            i1p = pi.tile([128, FD], F32, tag="i1p")
            for m in range(NCH):
                for c in range(NCH):
                    nc.tensor.matmul(
                        i1p[:, m * BC:(m + 1) * BC],
                        w1sb[:, c * H1 + m * 128: c * H1 + (m + 1) * 128],
                        S[0][:, c * BC:(c + 1) * BC],
                        start=(c == 0), stop=False)
                nc.tensor.matmul(
                    i1p[:, m * BC:(m + 1) * BC],
                    b1row[0:1, m * 128:(m + 1) * 128],
                    ones[0:1, :],
                    start=False, stop=True)
            return i1p

        # ---- the pipelined timestep loop -----------------------------
        rates(0)
        for k in range(T):
            rates(1)                       # ScalarE (runs during DVE layer-0)
            dve_block(0)                   # DVE layer 0 step k
            if debug == 2 and k == 0:
                srcs = dict(SMN=SMN, P1=P1, P2=P2, P3=P3,
                            ISUM=ISUM, V1T=V1T[0], IEXT=IEXT[0])
                for n, t in srcs.items():
                    fd = t.shape[-1]
                    DBGS = sb.tile([128, fd], F32, name=f"DBGS_{n}")
                    nc.vector.tensor_copy(DBGS[:], t[:])
                    nc.sync.dma_start(dbgs_d[n][:, :fd], DBGS[:])
            if k + 1 < T:
                rates(0)                   # ScalarE (runs during DVE layer-1)
            i1p = pe_i1(k)                 # PE
            dve_block(1, iext=i1p[:])      # DVE layer 1 step k (PSUM read)
            nc.tensor.matmul(accp[:], idsb[:], S[1][:],
                             start=(k == 0), stop=(k == T - 1),
                             skip_group_check=True)

        # ---- readout: (acc/T) @ w_out + b_out ------------------------
        nc.scalar.activation(RATE[:], accp[:], AF.Identity, bias=0.0, scale=1.0 / T)
        for c in range(NCH):
            nc.tensor.matmul(outp[:],
                             wosb[:, c * OUT:(c + 1) * OUT],
                             RATE[:, c * BC:(c + 1) * BC],
                             start=(c == 0), stop=(c == NCH - 1))
        nc.scalar.activation(OUTS[:], outp[:], AF.Identity, bias=bosb[:, 0:1], scale=1.0)
        nc.sync.dma_start(out_d[:], OUTS[:])

        if debug:
            dbg_src = [V[0][:], G[0][:, :FD], G[0][:, 2 * FD:], G[0][:, FD:2 * FD],
                       V[1][:], G[1][:, :FD], G[1][:, 2 * FD:], G[1][:, FD:2 * FD]]
            for i, sap in enumerate(dbg_src):
                DBG = sb.tile([128, FD], F32, name=f"DBG{i}")
                nc.vector.tensor_copy(DBG[:], sap)
                nc.sync.dma_start(dbg_d[i][:], DBG[:])
            nc.scalar.activation(RATE[:], accp[:], AF.Identity, bias=0.0, scale=1.0)
            nc.sync.dma_start(dbga_d[:], RATE[:])
    nc.compile()
    return nc


_NC_CACHE = {}


def _get_nc(T, scal, debug=False, b1_const=None):
    key = (T, tuple(sorted(scal.items())), debug, b1_const)
    if key not in _NC_CACHE:
        _NC_CACHE[key] = _build(T, scal, debug, b1_const=b1_const)
    return _NC_CACHE[key]


def _chunk_major(vec):
    """[1024] -> [128, 8] with vec[c*128+p] at [p, c]."""
    return np.ascontiguousarray(vec.reshape(NCH, 128).T)


def _make_in_maps(inputs, T, scal):
    gl = scal["g_leak"]; v_rest = scal["v_rest"]
    beta = DT * gl * v_rest

    x = np.asarray(inputs["x"], np.float32)
    w_exc0 = np.ascontiguousarray(np.asarray(inputs["w_exc0"], np.float32))
    W1 = np.concatenate([np.asarray(inputs["w_exc1"], np.float32),
                         -np.asarray(inputs["w_inh1"], np.float32)], axis=0)
    w1dt = (DT * W1).astype(ml_dtypes.bfloat16)
    b0dt = (_chunk_major(DT * np.asarray(inputs["b_exc0"], np.float32)) + beta
            ).astype(np.float32)
    b1row = (DT * (np.asarray(inputs["b_exc1"], np.float32)
                   - np.asarray(inputs["b_inh1"], np.float32)) + beta
             ).reshape(1, H1).astype(ml_dtypes.bfloat16)
    w_out = np.ascontiguousarray(np.asarray(inputs["w_out"], np.float32))
    b_out = np.asarray(inputs["b_out"], np.float32).reshape(128, 1)
    ident = np.eye(128, dtype=ml_dtypes.bfloat16)

    in_maps = []
    for c in range(NCORES):
        xT = np.ascontiguousarray(x[c * BC:(c + 1) * BC, :].T)
        in_maps.append({
            "xT": xT, "w_exc0": w_exc0, "b0dt": b0dt, "w1dt": w1dt,
            "b1row": b1row, "w_out": w_out, "b_out": b_out, "ident": ident,
        })
    return in_maps


def _b1_const(inputs, scal):
    """If b_exc1 - b_inh1 is a uniform constant, the PE bias rows can be
    dropped and DT*b1 + beta folded into the IEXT copy bias."""
    b1 = (np.asarray(inputs["b_exc1"], np.float32)
          - np.asarray(inputs["b_inh1"], np.float32))
    if np.all(b1 == b1.flat[0]):
        beta = DT * scal["g_leak"] * scal["v_rest"]
        return float(DT * b1.flat[0] + beta)
    return None


def kernel(**inputs):
    T = int(np.asarray(inputs["timesteps"]))
    scal = {k: float(np.asarray(inputs[k])) for k in
            ("v_rest", "v_threshold", "v_reset", "g_na_max", "g_k_max",
             "g_leak", "e_na", "e_k")}
    nc = _get_nc(T, scal, b1_const=_b1_const(inputs, scal))
    in_maps = _make_in_maps(inputs, T, scal)
    res = run_bass_kernel_spmd(nc, in_maps, core_ids=list(range(NCORES)))
    out = np.empty((B, OUT), np.float32)
    for c in range(NCORES):
        out[c * BC:(c + 1) * BC, :] = res.results[c]["out"].T
    return out


# revision 15
# speedup vs baseline: 1.0526x; 1.0012x over previous
"""Trainium2 Bass kernel for the EnhancedNeuromorphicNetwork HH spiking net.

Strategy (pure batch data-parallel across 8 cores, B=512 -> 64 rows/core):
  - All HH state lives in SBUF, per-layer tiles [128 part, 512 free] in
    chunk-major layout: state[p, c*64 + b] for neuron j = c*128+p.
  - The two layers are pipelined: ScalarE computes the 6 transcendental
    rate activations for one layer while the DVE runs the gate/current/
    voltage update of the other layer, so both engines stay busy.
  - Three custom DVE ops fuse the hot math:
      HH_RATE : (v + K) * recipNR(1 - e)      (alpha_m / alpha_n rational)
      HH_M3H  : m^3 h
      HH_N4V  : n^4 * (v - e_k) * (DT g_k)
    plus scalar_tensor_tensor fusions for the gate updates
      g' = DT*a - (sigma - 1) * g,  sigma = DT*(a+b).
  - Layer-1 input current i1 = s0 @ (DT*W1) runs on the tensor engine with
    an extra K=1 "ones" row folding DT*b1 + leak-beta into the PSUM, so a
    single ScalarE copy materializes IEXT.
  - Spike accumulation acc += s1 is an identity-matmul into persistent PSUM.

The final output is (acc/T) @ w_out + b_out computed on-device, gathered
per-core as [OUT=128, 64] and reassembled on host.
"""
import math
from contextlib import ExitStack

import ml_dtypes
import numpy as np

import concourse.bacc as bacc
import concourse.bass as bass
import concourse.mybir as mybir
import concourse.tile as tile
from concourse.bass_utils import run_bass_kernel_spmd

DT = 0.1
B, IN, H0, H1, OUT = 512, 512, 1024, 1024, 128
E0 = int(0.8 * H0)
NCORES = 8
BC = B // NCORES          # batch per core (64)
KC0 = IN // 128           # K chunks for the input matmul (4)
NCH = H0 // 128           # H chunks (8)
FD = NCH * BC             # free dim per layer (512)

F32 = mybir.dt.float32
BF16 = mybir.dt.bfloat16
AF = mybir.ActivationFunctionType
ALU = mybir.AluOpType

# Chebyshev-minimax seed constants (same interval as RECIPROCAL_APPROX_FAST)
_RC0 = -0.23549792
_RC1 = 2.0017324
# sqrt(lambda) output-scale folds for the NR reciprocal (out = lambda/d)
_SQLM = math.sqrt(0.1 * DT)     # lambda_m = 0.1*DT  -> AMN_m = DT*a_m
_SQLN = math.sqrt(0.01 * DT)    # lambda_n = 0.01*DT -> AMN_n = DT*a_n


def _register_ops():
    """Register the three fused custom-DVE ops used by the HH update.

    HH_RATE_V : out = (in1 + imm2) * recip1NR(1 - in0)
                (one exponent-flip seed + one Newton-Raphson pass)
    HH_M3H    : out = in0^3 * in1
    HH_N4V    : out = in0^4 * (in1 - s0) * s1
    """
    from concourse import dve_ops as dvo
    from concourse.dve_spec import Spec, Src0, Src1, C0, C1, C2, One, Bin, AluOp
    from concourse.dve_spec import lower as dve_lower
    from concourse.dve_uop import DveOpSpec

    def reg(name, spec):
        for op in dvo.OPS:
            if op.name == name:
                return op
        shas = {}
        for ver in ("v3", "v4"):
            uops = dve_lower(spec, ver=ver)
            shas[ver] = DveOpSpec(name=name, opcode=0, uops=uops, rd1_en=True).sha(ver)
        op = dvo.DveOp(name, spec, subdim=False, uops_sha=shas)
        dvo.OPS.append(op)
        dvo.CUSTOM_DVE_SPECS[name] = spec
        dvo._SUB_OPCODE_FOR_NAME[name] = max(dvo._SUB_OPCODE_FOR_NAME.values()) + 1
        assert dvo._SUB_OPCODE_FOR_NAME[name] < 0x20
        return op

    def np_not(x):
        return (~np.asarray(x, np.float32).view(np.int32)).view(np.float32)

    # rate: d = 1 - in0 ; y0 = NOT(d)*s0 ; y1 = y0*(s1 - d*y0) ; out = in1*y1
    # (in1 carries the pre-scaled numerator, e.g. 0.01*(v+40))
    d = One - Src0
    nd = Bin(AluOp.BITWISE_NOT, d, d)
    y0 = nd * C0
    rate = Spec(
        body=Src1 * (y0 * (C1 - d * y0)),
        reference=lambda in0, in1, s0, s1, imm2: (
            (lambda dd, yy0: in1 * (yy0 * (s1 - dd * yy0)))(
                (1.0 - in0).astype(np.float32),
                np_not((1.0 - in0).astype(np.float32)) * np.float32(s0),
            )
        ),
    )
    m3h = Spec(
        body=((Src0 * Src0) * (Src0 * Src1)) * C0,
        reference=lambda in0, in1, s0, s1, imm2: (
            (in0 * in0) * (in0 * in1) * np.float32(s0)
        ),
    )
    sq_n = Src0 * Src0
    n4v = Spec(
        body=(sq_n * sq_n) * ((Src1 - C0) * C1),
        reference=lambda in0, in1, s0, s1, imm2: (
            (in0 * in0) * (in0 * in0) * ((in1 - np.float32(s0)) * np.float32(s1))
        ),
    )
    return reg("HH_RATE_V", rate), reg("HH_M3H", m3h), reg("HH_N4V", n4v)


def _build(T, scal, debug=False, b1_const=None):
    """Build the SPMD Bass module for `T` timesteps.

    scal: dict of python-float HH parameters (folded into immediates).
    """
    v_rest = scal["v_rest"]; v_th = scal["v_threshold"]; v_res = scal["v_reset"]
    gna = scal["g_na_max"]; gk = scal["g_k_max"]; gl = scal["g_leak"]
    ena = scal["e_na"]; ek = scal["e_k"]
    alpha = 1.0 - DT * gl          # leak folded into the v update
    # beta ( = DT*gl*v_rest ) is folded into the IEXT tiles host/bias side.
    ln = math.log

    op_rate, op_m3h, op_n4v = _register_ops()

    nc = bacc.Bacc()
    xT_d = nc.declare_dram_parameter("xT", [IN, BC], F32, isOutput=False)
    w0_d = nc.declare_dram_parameter("w_exc0", [IN, H0], F32, isOutput=False)
    b0_d = nc.declare_dram_parameter("b0dt", [128, NCH], F32, isOutput=False)
    w1_d = nc.declare_dram_parameter("w1dt", [H0, H1], BF16, isOutput=False)
    b1r_d = nc.declare_dram_parameter("b1row", [1, H1], BF16, isOutput=False)
    wo_d = nc.declare_dram_parameter("w_out", [H1, OUT], F32, isOutput=False)
    bo_d = nc.declare_dram_parameter("b_out", [128, 1], F32, isOutput=False)
    id_d = nc.declare_dram_parameter("ident", [128, 128], BF16, isOutput=False)
    out_d = nc.declare_dram_parameter("out", [OUT, BC], F32, isOutput=True)
    if debug:
        dbg_d = [nc.declare_dram_parameter(f"dbg{i}", [128, FD], F32, isOutput=True)
                 for i in range(8)]
        dbga_d = nc.declare_dram_parameter("dbg_acc", [128, FD], F32, isOutput=True)
    if debug == 2:
        dbgs_d = {n: nc.declare_dram_parameter(f"dbgs_{n}", [128, 2 * FD], F32,
                                               isOutput=True)
                  for n in ("E1", "E2", "AH", "TH", "AMN", "SMN", "UMN",
                            "P1", "P2", "P3", "ISUM", "SH1", "V1T", "IEXT")}

    with tile.TileContext(nc) as tc, ExitStack() as ctx:
        sb = ctx.enter_context(tc.tile_pool(name="sb", bufs=1))
        pp = ctx.enter_context(tc.tile_pool(name="pp", bufs=1, space="PSUM"))
        pi = ctx.enter_context(tc.tile_pool(name="pi", bufs=2, space="PSUM"))

        # ---- persistent SBUF tiles -----------------------------------
        w1sb = sb.tile([128, NCH * H1], BF16)        # DT*W1 chunk-major
        w0sb = sb.tile([128, KC0 * H0], F32)
        wosb = sb.tile([128, NCH * OUT], F32)
        xtsb = sb.tile([128, KC0 * BC], F32)
        b0sb = sb.tile([128, NCH], F32)
        b1row = sb.tile([1, H1], BF16)               # DT*b1 + beta
        bosb = sb.tile([128, 1], F32)
        idsb = sb.tile([128, 128], BF16)
        ones = sb.tile([1, BC], BF16)

        # per-layer state: gates [m|n|h] paired in one [128,1536] tile
        V = [sb.tile([128, FD], BF16, name=f"V{L}") for L in range(2)]
        G = [sb.tile([128, 3 * FD], BF16, name=f"G{L}") for L in range(2)]
        S = [sb.tile([128, FD], BF16, name=f"S{L}") for L in range(2)]
        # per-layer rate-activation outputs (ScalarE); A holds [DT*am|DT*an|DT*ah]
        E12 = [sb.tile([128, 2 * FD], F32, name=f"E12_{L}") for L in range(2)]
        VOFF = [sb.tile([128, 2 * FD], BF16, name=f"VOFF{L}") for L in range(2)]
        A = [sb.tile([128, 3 * FD], BF16, name=f"A{L}") for L in range(2)]
        BMN = [sb.tile([128, 2 * FD], BF16, name=f"BMN{L}") for L in range(2)]
        TH = [sb.tile([128, FD], BF16, name=f"TH{L}") for L in range(2)]
        TH2 = [sb.tile([128, FD], BF16, name=f"TH2_{L}") for L in range(2)]
        IEXT = [sb.tile([128, FD], BF16, name=f"IEXT{L}") for L in range(2)]
        V1T = [sb.tile([128, FD], BF16, name=f"V1T{L}") for L in range(2)]
        # shared scratch (written+consumed inside one DVE block)
        SMN = sb.tile([128, 2 * FD], BF16)
        SGA = sb.tile([128, 3 * FD], BF16)   # [sigma-1] for m,n,h
        UG = sb.tile([128, 3 * FD], BF16)
        P1 = sb.tile([128, FD], BF16)
        P2 = sb.tile([128, FD], BF16)
        P3 = sb.tile([128, FD], BF16)
        ISUM = sb.tile([128, FD], BF16)
        VRST = sb.tile([128, FD], BF16)
        RATE = sb.tile([128, FD], F32)
        OUTS = sb.tile([128, BC], F32)

        BIASC = sb.tile([128, 10], F32)      # activation bias constants

        accp = pp.tile([128, FD], F32)
        i0p = pp.tile([128, FD], F32)
        outp = pp.tile([128, BC], F32)

        # ---- loads (one DMA per tensor; chunk-major into partitions) --
        # layer-0 input path first: the first DVE block waits on IEXT[0]
        nc.sync.dma_start(xtsb[:].rearrange("p (c n) -> p c n", c=KC0),
                          xT_d[:].rearrange("(c p) n -> p c n", p=128))
        nc.sync.dma_start(b0sb[:], b0_d[:])
        nc.sync.dma_start(w0sb[:].rearrange("p (c m) -> p c m", c=KC0),
                          w0_d[:].rearrange("(c p) m -> p c m", p=128))
        nc.sync.dma_start(w1sb[:].rearrange("p (c m) -> p c m", c=NCH),
                          w1_d[:].rearrange("(c p) m -> p c m", p=128))
        nc.sync.dma_start(b1row[:], b1r_d[:])
        nc.sync.dma_start(idsb[:], id_d[:])
        nc.sync.dma_start(wosb[:].rearrange("p (c o) -> p c o", c=NCH),
                          wo_d[:].rearrange("(c p) o -> p c o", p=128))
        nc.sync.dma_start(bosb[:], bo_d[:])

        # ---- init -----------------------------------------------------
        for L in range(2):
            nc.vector.memset(V[L][:], v_rest)
            nc.vector.memset(G[L][:, :FD], 0.05)
            nc.vector.memset(G[L][:, FD:2 * FD], 0.32)
            nc.vector.memset(G[L][:, 2 * FD:], 0.6)
        nc.vector.memset(VRST[:], v_res)
        nc.gpsimd.memset(ones[:], 1.0)
        bias_vals = [-4.0,                                  # E1
                     -5.5,                                  # E2
                     -65.0 / 20.0 + ln(0.07 * DT),          # AH
                     -65.0 / 18.0 + ln(4.0 * DT),          # BM
                     -65.0 / 80.0 + ln(0.125 * DT),         # BN
                     35.0 / 20.0,                           # TH
                     DT / 2.0 - 1.0,                        # TH2
                     0.0 if b1_const is None else b1_const, # IEXT1 bias
                     0.4,                                   # VOFFm: .01*(v+40)
                     0.055]                                 # VOFFn: .001*(v+55)
        for i, bv in enumerate(bias_vals):
            nc.gpsimd.memset(BIASC[:, i:i + 1], bv)
        (bE1, bE2, bAH, bBM, bBN, bTH, bTH2, bB1, bVm, bVn) = (
            BIASC[:, i:i + 1] for i in range(10))

        # i0 = x_shard @ w_exc0 ;  IEXT[0] = (DT/T)*psum + DT*b0 + beta
        for m in range(NCH):
            for c in range(KC0):
                nc.tensor.matmul(
                    i0p[:, m * BC:(m + 1) * BC],
                    w0sb[:, c * H0 + m * 128: c * H0 + (m + 1) * 128],
                    xtsb[:, c * BC:(c + 1) * BC],
                    start=(c == 0), stop=(c == KC0 - 1))
        for m in range(NCH):
            nc.scalar.activation(IEXT[0][:, m * BC:(m + 1) * BC],
                                 i0p[:, m * BC:(m + 1) * BC],
                                 AF.Identity, bias=b0sb[:, m:m + 1],
                                 scale=DT / T)

        # ---- per-step building blocks --------------------------------
        def rates(L):
            """ScalarE: rate tiles for layer L (reads V[L])."""
            v = V[L][:]
            nc.scalar.activation(E12[L][:, :FD], v, AF.Exp, bias=bE1, scale=-0.1)
            nc.scalar.activation(E12[L][:, FD:], v, AF.Exp, bias=bE2, scale=-0.1)
            nc.scalar.activation(VOFF[L][:, :FD], v, AF.Identity, bias=bVm,
                                 scale=0.01)
            nc.scalar.activation(VOFF[L][:, FD:], v, AF.Identity, bias=bVn,
                                 scale=0.001)
            nc.scalar.activation(BMN[L][:, :FD], v, AF.Exp, bias=bBM,
                                 scale=-1.0 / 18.0)
            nc.scalar.activation(BMN[L][:, FD:], v, AF.Exp, bias=bBN,
                                 scale=-1.0 / 80.0)
            nc.scalar.activation(TH[L][:], v, AF.Tanh, bias=bTH, scale=1.0 / 20.0)
            nc.scalar.activation(TH2[L][:], TH[L][:], AF.Identity, bias=bTH2,
                                 scale=DT / 2.0)
            nc.scalar.activation(A[L][:, 2 * FD:], v, AF.Exp, bias=bAH,
                                 scale=-1.0 / 20.0)

        def dve_block(L, iext=None):
            """DVE: full HH update for layer L (gates, currents, v, spike)."""
            v = V[L][:]
            iext = IEXT[L][:] if iext is None else iext
            g = G[L][:]
            stt = nc.vector.scalar_tensor_tensor
            tt = nc.vector.tensor_tensor
            # rational rates in one [1024] custom: A[:2FD] = [DT*a_m | DT*a_n]
            nc.vector._custom_dve(op_rate, out=A[L][:, :2 * FD],
                                  in0=E12[L][:], in1=VOFF[L][:],
                                  s0=_RC0, s1=_RC1, imm2=0.0)
            # sigma-1 for all gates -> SGA [m|n|h]
            tt(SMN[:], A[L][:, :2 * FD], BMN[L][:], ALU.add)
            nc.vector.tensor_scalar(SGA[:, :2 * FD], SMN[:], 1.0, None,
                                    ALU.subtract)
            tt(SGA[:, 2 * FD:], TH2[L][:], A[L][:, 2 * FD:], ALU.add)
            # fused 3-gate update: g' = DT*a - (sigma-1)*g
            tt(UG[:], SGA[:], g, ALU.mult)
            tt(g, A[L][:], UG[:], ALU.subtract)
            # currents: P1 = DT*gna*m^3 h ; P2 = DT*gk*n^4*(v-ek)
            nc.vector._custom_dve(op_m3h, out=P1[:], in0=G[L][:, :FD],
                                  in1=G[L][:, 2 * FD:], s0=DT * gna, s1=0.0,
                                  imm2=0.0)
            nc.vector._custom_dve(op_n4v, out=P2[:], in0=G[L][:, FD:2 * FD],
                                  in1=v, s0=ek, s1=DT * gk, imm2=0.0)
            stt(P3[:], v, ena, P1[:], ALU.subtract, ALU.mult)
            tt(ISUM[:], P3[:], P2[:], ALU.add)
            # v' = (alpha*v - ISUM) + IEXT ; only the last (short) op
            # depends on IEXT, which for layer 1 arrives late via PE+copy
            stt(V1T[L][:], v, alpha, ISUM[:], ALU.mult, ALU.subtract)
            tt(v, V1T[L][:], iext, ALU.add)
            # spike + reset (bf16 spike tile doubles as the int mask)
            nc.vector.tensor_scalar(S[L][:], v, v_th, None, ALU.is_gt)
            nc.vector.copy_predicated(v, S[L][:].bitcast(mybir.dt.uint16),
                                      VRST[:])

        def pe_i1(k):
            """i1 = s0 @ (DT*W1) + (DT*b1 + beta) into fresh PSUM tile."""
            i1p = pi.tile([128, FD], F32, tag="i1p")
            for m in range(NCH):
                for c in range(NCH):
                    nc.tensor.matmul(
                        i1p[:, m * BC:(m + 1) * BC],
                        w1sb[:, c * H1 + m * 128: c * H1 + (m + 1) * 128],
                        S[0][:, c * BC:(c + 1) * BC],
                        start=(c == 0), stop=False)
                nc.tensor.matmul(
                    i1p[:, m * BC:(m + 1) * BC],
                    b1row[0:1, m * 128:(m + 1) * 128],
                    ones[0:1, :],
                    start=False, stop=True)
            return i1p

        # ---- the pipelined timestep loop -----------------------------
        rates(0)
        for k in range(T):
            rates(1)                       # ScalarE (runs during DVE layer-0)
            dve_block(0)                   # DVE layer 0 step k
            if debug == 2 and k == 0:
                srcs = dict(SMN=SMN, P1=P1, P2=P2, P3=P3,
                            ISUM=ISUM, V1T=V1T[0], IEXT=IEXT[0])
                for n, t in srcs.items():
                    fd = t.shape[-1]
                    DBGS = sb.tile([128, fd], F32, name=f"DBGS_{n}")
                    nc.vector.tensor_copy(DBGS[:], t[:])
                    nc.sync.dma_start(dbgs_d[n][:, :fd], DBGS[:])
            if k + 1 < T:
                rates(0)                   # ScalarE (runs during DVE layer-1)
            i1p = pe_i1(k)                 # PE
            dve_block(1, iext=i1p[:])      # DVE layer 1 step k (PSUM read)
            nc.tensor.matmul(accp[:], idsb[:], S[1][:],
                             start=(k == 0), stop=(k == T - 1),
                             skip_group_check=True)

        # ---- readout: (acc/T) @ w_out + b_out ------------------------
        nc.scalar.activation(RATE[:], accp[:], AF.Identity, bias=0.0, scale=1.0 / T)
        for c in range(NCH):
            nc.tensor.matmul(outp[:],
                             wosb[:, c * OUT:(c + 1) * OUT],
                             RATE[:, c * BC:(c + 1) * BC],
                             start=(c == 0), stop=(c == NCH - 1))
        nc.scalar.activation(OUTS[:], outp[:], AF.Identity, bias=bosb[:, 0:1], scale=1.0)
        nc.sync.dma_start(out_d[:], OUTS[:])

        if debug:
            dbg_src = [V[0][:], G[0][:, :FD], G[0][:, 2 * FD:], G[0][:, FD:2 * FD],
                       V[1][:], G[1][:, :FD], G[1][:, 2 * FD:], G[1][:, FD:2 * FD]]
            for i, sap in enumerate(dbg_src):
                DBG = sb.tile([128, FD], F32, name=f"DBG{i}")
                nc.vector.tensor_copy(DBG[:], sap)
                nc.sync.dma_start(dbg_d[i][:], DBG[:])
            nc.scalar.activation(RATE[:], accp[:], AF.Identity, bias=0.0, scale=1.0)
            nc.sync.dma_start(dbga_d[:], RATE[:])
    nc.compile()
    return nc


_NC_CACHE = {}


def _get_nc(T, scal, debug=False, b1_const=None):
    key = (T, tuple(sorted(scal.items())), debug, b1_const)
    if key not in _NC_CACHE:
        _NC_CACHE[key] = _build(T, scal, debug, b1_const=b1_const)
    return _NC_CACHE[key]


def _chunk_major(vec):
    """[1024] -> [128, 8] with vec[c*128+p] at [p, c]."""
    return np.ascontiguousarray(vec.reshape(NCH, 128).T)


def _make_in_maps(inputs, T, scal):
    gl = scal["g_leak"]; v_rest = scal["v_rest"]
    beta = DT * gl * v_rest

    x = np.asarray(inputs["x"], np.float32)
    w_exc0 = np.ascontiguousarray(np.asarray(inputs["w_exc0"], np.float32))
    W1 = np.concatenate([np.asarray(inputs["w_exc1"], np.float32),
                         -np.asarray(inputs["w_inh1"], np.float32)], axis=0)
    w1dt = (DT * W1).astype(ml_dtypes.bfloat16)
    b0dt = (_chunk_major(DT * np.asarray(inputs["b_exc0"], np.float32)) + beta
            ).astype(np.float32)
    b1row = (DT * (np.asarray(inputs["b_exc1"], np.float32)
                   - np.asarray(inputs["b_inh1"], np.float32)) + beta
             ).reshape(1, H1).astype(ml_dtypes.bfloat16)
    w_out = np.ascontiguousarray(np.asarray(inputs["w_out"], np.float32))
    b_out = np.asarray(inputs["b_out"], np.float32).reshape(128, 1)
    ident = np.eye(128, dtype=ml_dtypes.bfloat16)

    in_maps = []
    for c in range(NCORES):
        xT = np.ascontiguousarray(x[c * BC:(c + 1) * BC, :].T)
        in_maps.append({
            "xT": xT, "w_exc0": w_exc0, "b0dt": b0dt, "w1dt": w1dt,
            "b1row": b1row, "w_out": w_out, "b_out": b_out, "ident": ident,
        })
    return in_maps


def _b1_const(inputs, scal):
    """If b_exc1 - b_inh1 is a uniform constant, the PE bias rows can be
    dropped and DT*b1 + beta folded into the IEXT copy bias."""
    b1 = (np.asarray(inputs["b_exc1"], np.float32)
          - np.asarray(inputs["b_inh1"], np.float32))
    if np.all(b1 == b1.flat[0]):
        beta = DT * scal["g_leak"] * scal["v_rest"]
        return float(DT * b1.flat[0] + beta)
    return None


def kernel(**inputs):
    T = int(np.asarray(inputs["timesteps"]))
    scal = {k: float(np.asarray(inputs[k])) for k in
            ("v_rest", "v_threshold", "v_reset", "g_na_max", "g_k_max",
             "g_leak", "e_na", "e_k")}
    nc = _get_nc(T, scal, b1_const=_b1_const(inputs, scal))
    in_maps = _make_in_maps(inputs, T, scal)
    res = run_bass_kernel_spmd(nc, in_maps, core_ids=list(range(NCORES)))
    out = np.empty((B, OUT), np.float32)
    for c in range(NCORES):
        out[c * BC:(c + 1) * BC, :] = res.results[c]["out"].T
    return out


# revision 16
# speedup vs baseline: 1.0543x; 1.0016x over previous
"""Trainium2 Bass kernel for the EnhancedNeuromorphicNetwork HH spiking net.

Strategy (pure batch data-parallel across 8 cores, B=512 -> 64 rows/core):
  - All HH state lives in SBUF, per-layer tiles [128 part, 512 free] in
    chunk-major layout: state[p, c*64 + b] for neuron j = c*128+p.
  - The two layers are pipelined: ScalarE computes the 6 transcendental
    rate activations for one layer while the DVE runs the gate/current/
    voltage update of the other layer, so both engines stay busy.
  - Three custom DVE ops fuse the hot math:
      HH_RATE : (v + K) * recipNR(1 - e)      (alpha_m / alpha_n rational)
      HH_M3H  : m^3 h
      HH_N4V  : n^4 * (v - e_k) * (DT g_k)
    plus scalar_tensor_tensor fusions for the gate updates
      g' = DT*a - (sigma - 1) * g,  sigma = DT*(a+b).
  - Layer-1 input current i1 = s0 @ (DT*W1) runs on the tensor engine with
    an extra K=1 "ones" row folding DT*b1 + leak-beta into the PSUM, so a
    single ScalarE copy materializes IEXT.
  - Spike accumulation acc += s1 is an identity-matmul into persistent PSUM.

The final output is (acc/T) @ w_out + b_out computed on-device, gathered
per-core as [OUT=128, 64] and reassembled on host.
"""
import math
from contextlib import ExitStack

import ml_dtypes
import numpy as np

import concourse.bacc as bacc
import concourse.bass as bass
import concourse.mybir as mybir
import concourse.tile as tile
from concourse.bass_utils import run_bass_kernel_spmd

DT = 0.1
B, IN, H0, H1, OUT = 512, 512, 1024, 1024, 128
E0 = int(0.8 * H0)
NCORES = 8
BC = B // NCORES          # batch per core (64)
KC0 = IN // 128           # K chunks for the input matmul (4)
NCH = H0 // 128           # H chunks (8)
FD = NCH * BC             # free dim per layer (512)

F32 = mybir.dt.float32
BF16 = mybir.dt.bfloat16
AF = mybir.ActivationFunctionType
ALU = mybir.AluOpType

# Chebyshev-minimax seed constants (same interval as RECIPROCAL_APPROX_FAST)
_RC0 = -0.23549792
_RC1 = 2.0017324
# sqrt(lambda) output-scale folds for the NR reciprocal (out = lambda/d)
_SQLM = math.sqrt(0.1 * DT)     # lambda_m = 0.1*DT  -> AMN_m = DT*a_m
_SQLN = math.sqrt(0.01 * DT)    # lambda_n = 0.01*DT -> AMN_n = DT*a_n


def _register_ops():
    """Register the three fused custom-DVE ops used by the HH update.

    HH_RATE_V : out = (in1 + imm2) * recip1NR(1 - in0)
                (one exponent-flip seed + one Newton-Raphson pass)
    HH_M3H    : out = in0^3 * in1
    HH_N4V    : out = in0^4 * (in1 - s0) * s1
    """
    from concourse import dve_ops as dvo
    from concourse.dve_spec import Spec, Src0, Src1, C0, C1, C2, One, Bin, AluOp
    from concourse.dve_spec import lower as dve_lower
    from concourse.dve_uop import DveOpSpec

    def reg(name, spec):
        for op in dvo.OPS:
            if op.name == name:
                return op
        shas = {}
        for ver in ("v3", "v4"):
            uops = dve_lower(spec, ver=ver)
            shas[ver] = DveOpSpec(name=name, opcode=0, uops=uops, rd1_en=True).sha(ver)
        op = dvo.DveOp(name, spec, subdim=False, uops_sha=shas)
        dvo.OPS.append(op)
        dvo.CUSTOM_DVE_SPECS[name] = spec
        dvo._SUB_OPCODE_FOR_NAME[name] = max(dvo._SUB_OPCODE_FOR_NAME.values()) + 1
        assert dvo._SUB_OPCODE_FOR_NAME[name] < 0x20
        return op

    def np_not(x):
        return (~np.asarray(x, np.float32).view(np.int32)).view(np.float32)

    # rate: d = 1 - in0 ; y0 = NOT(d)*s0 ; y1 = y0*(s1 - d*y0) ; out = in1*y1
    # (in1 carries the pre-scaled numerator, e.g. 0.01*(v+40))
    d = One - Src0
    nd = Bin(AluOp.BITWISE_NOT, d, d)
    y0 = nd * C0
    rate = Spec(
        body=Src1 * (y0 * (C1 - d * y0)),
        reference=lambda in0, in1, s0, s1, imm2: (
            (lambda dd, yy0: in1 * (yy0 * (s1 - dd * yy0)))(
                (1.0 - in0).astype(np.float32),
                np_not((1.0 - in0).astype(np.float32)) * np.float32(s0),
            )
        ),
    )
    m3h = Spec(
        body=((Src0 * Src0) * (Src0 * Src1)) * C0,
        reference=lambda in0, in1, s0, s1, imm2: (
            (in0 * in0) * (in0 * in1) * np.float32(s0)
        ),
    )
    sq_n = Src0 * Src0
    n4v = Spec(
        body=(sq_n * sq_n) * ((Src1 - C0) * C1),
        reference=lambda in0, in1, s0, s1, imm2: (
            (in0 * in0) * (in0 * in0) * ((in1 - np.float32(s0)) * np.float32(s1))
        ),
    )
    return reg("HH_RATE_V", rate), reg("HH_M3H", m3h), reg("HH_N4V", n4v)




def _install_m3h_2x():
    """Hand-authored 2x_1p uop for HH_M3H: blocks 0-3 compute the lo bf16
    element, blocks 4-7 the hi element; the lo result rides delay chain 0
    to WR0_LO while blk7's ALU feeds WR0_HI. Validated on HW vs numpy."""
    from concourse import dve_ops as dvo
    from concourse.dve_spec import lower as dve_lower
    from concourse.dve_ops import has_src1
    from concourse.dve_uop import (AluInp, AluOp, DelayInp, DveOpSpec, InpSel,
                                   OutPath, OutSel, Trigger, UopConfig,
                                   UopDpConfig)
    ver = "v3"
    if ("HH_M3H", ver) in dvo._COMPILE_CACHE:
        return
    PD, PA = DelayInp.PREV_DELAY, DelayInp.PREV_ALU_OUT

    def blk(op=None, s0=None, s1=None, loads=(), passes=()):
        b = UopDpConfig()
        if op is not None:
            b.enable_alu(op, s0, s1)
        else:
            b.pass_through_alu()
        for c, srcd in loads:
            b.enable_delay_from_src(srcd, c)
        for c in passes:
            b.pass_through_delay(c)
        return b

    u = UopConfig()
    u.inp = [InpSel.ZERO, InpSel.SRC_0, InpSel.SRC_1, InpSel.CONST_0,
             InpSel.SRC_0_HI, InpSel.SRC_1_HI, InpSel.ZERO, InpSel.ZERO]
    u.inp_enable = [0, 1, 1, 1, 1, 1, 0, 0]
    D, M = AluInp, AluOp.MULTIPLY
    u.datapath_config = [
        blk(M, D.PREV_DELAY_0, D.PREV_DELAY_0,
            loads=[(0, PD), (1, PD), (2, PD), (3, PD), (4, PD)]),
        blk(M, D.PREV_DELAY_0, D.PREV_DELAY_1, loads=[(0, PA)], passes=(2, 3, 4)),
        blk(M, D.PREV_DELAY_0, D.PREV_ALU_OUT, passes=(2, 3, 4)),
        blk(M, D.PREV_ALU_OUT, D.PREV_DELAY_2, passes=(2, 3, 4)),
        blk(M, D.PREV_DELAY_3, D.PREV_DELAY_3, loads=[(0, PA)], passes=(2, 3, 4)),
        blk(M, D.PREV_DELAY_3, D.PREV_DELAY_4, loads=[(1, PA)], passes=(0, 2)),
        blk(M, D.PREV_DELAY_1, D.PREV_ALU_OUT, passes=(0, 2)),
        blk(M, D.PREV_ALU_OUT, D.PREV_DELAY_2, passes=(0,)),
    ]
    u.out = {OutPath.WR0_LO: OutSel.DELAY_0, OutPath.WR0_HI: OutSel.ALU_OUT,
             OutPath.WR1_LO: OutSel.ALU_OUT, OutPath.WR1_HI: OutSel.ALU_OUT}
    u.out_enable = {OutPath.WR0_LO: 1, OutPath.WR0_HI: 1,
                    OutPath.WR1_LO: 0, OutPath.WR1_HI: 0}
    u.require_inp0 = 1
    u.require_inp1 = 1
    u.trigger = (Trigger.SRC_TENSOR_DONE, Trigger.NONE, Trigger.NONE)
    u.next_uop = (0, 0, 0)
    u.validate(ver)
    op = next(o for o in dvo.OPS if o.name == "HH_M3H")
    dvo._COMPILE_CACHE[("HH_M3H", ver)] = DveOpSpec(
        name=op.name, opcode=dvo.get_dve_sub_opcode(op.name),
        uops=dve_lower(op.spec, ver=ver), uops_2x=[u],
        rd1_en=has_src1(op.spec))

def _build(T, scal, debug=False, b1_const=None):
    """Build the SPMD Bass module for `T` timesteps.

    scal: dict of python-float HH parameters (folded into immediates).
    """
    v_rest = scal["v_rest"]; v_th = scal["v_threshold"]; v_res = scal["v_reset"]
    gna = scal["g_na_max"]; gk = scal["g_k_max"]; gl = scal["g_leak"]
    ena = scal["e_na"]; ek = scal["e_k"]
    alpha = 1.0 - DT * gl          # leak folded into the v update
    # beta ( = DT*gl*v_rest ) is folded into the IEXT tiles host/bias side.
    ln = math.log

    op_rate, op_m3h, op_n4v = _register_ops()
    _install_m3h_2x()

    nc = bacc.Bacc()
    xT_d = nc.declare_dram_parameter("xT", [IN, BC], F32, isOutput=False)
    w0_d = nc.declare_dram_parameter("w_exc0", [IN, H0], F32, isOutput=False)
    b0_d = nc.declare_dram_parameter("b0dt", [128, NCH], F32, isOutput=False)
    w1_d = nc.declare_dram_parameter("w1dt", [H0, H1], BF16, isOutput=False)
    b1r_d = nc.declare_dram_parameter("b1row", [1, H1], BF16, isOutput=False)
    wo_d = nc.declare_dram_parameter("w_out", [H1, OUT], F32, isOutput=False)
    bo_d = nc.declare_dram_parameter("b_out", [128, 1], F32, isOutput=False)
    id_d = nc.declare_dram_parameter("ident", [128, 128], BF16, isOutput=False)
    out_d = nc.declare_dram_parameter("out", [OUT, BC], F32, isOutput=True)
    if debug:
        dbg_d = [nc.declare_dram_parameter(f"dbg{i}", [128, FD], F32, isOutput=True)
                 for i in range(8)]
        dbga_d = nc.declare_dram_parameter("dbg_acc", [128, FD], F32, isOutput=True)
    if debug == 2:
        dbgs_d = {n: nc.declare_dram_parameter(f"dbgs_{n}", [128, 2 * FD], F32,
                                               isOutput=True)
                  for n in ("E1", "E2", "AH", "TH", "AMN", "SMN", "UMN",
                            "P1", "P2", "P3", "ISUM", "SH1", "V1T", "IEXT")}

    with tile.TileContext(nc) as tc, ExitStack() as ctx:
        sb = ctx.enter_context(tc.tile_pool(name="sb", bufs=1))
        pp = ctx.enter_context(tc.tile_pool(name="pp", bufs=1, space="PSUM"))
        pi = ctx.enter_context(tc.tile_pool(name="pi", bufs=2, space="PSUM"))

        # ---- persistent SBUF tiles -----------------------------------
        w1sb = sb.tile([128, NCH * H1], BF16)        # DT*W1 chunk-major
        w0sb = sb.tile([128, KC0 * H0], F32)
        wosb = sb.tile([128, NCH * OUT], F32)
        xtsb = sb.tile([128, KC0 * BC], F32)
        b0sb = sb.tile([128, NCH], F32)
        b1row = sb.tile([1, H1], BF16)               # DT*b1 + beta
        bosb = sb.tile([128, 1], F32)
        idsb = sb.tile([128, 128], BF16)
        ones = sb.tile([1, BC], BF16)

        # per-layer state: gates [m|n|h] paired in one [128,1536] tile
        V = [sb.tile([128, FD], BF16, name=f"V{L}") for L in range(2)]
        G = [sb.tile([128, 3 * FD], BF16, name=f"G{L}") for L in range(2)]
        S = [sb.tile([128, FD], BF16, name=f"S{L}") for L in range(2)]
        # per-layer rate-activation outputs (ScalarE); A holds [DT*am|DT*an|DT*ah]
        E12 = [sb.tile([128, 2 * FD], F32, name=f"E12_{L}") for L in range(2)]
        VOFF = [sb.tile([128, 2 * FD], BF16, name=f"VOFF{L}") for L in range(2)]
        A = [sb.tile([128, 3 * FD], BF16, name=f"A{L}") for L in range(2)]
        BMN = [sb.tile([128, 2 * FD], BF16, name=f"BMN{L}") for L in range(2)]
        TH = [sb.tile([128, FD], BF16, name=f"TH{L}") for L in range(2)]
        TH2 = [sb.tile([128, FD], BF16, name=f"TH2_{L}") for L in range(2)]
        IEXT = [sb.tile([128, FD], BF16, name=f"IEXT{L}") for L in range(2)]
        V1T = [sb.tile([128, FD], BF16, name=f"V1T{L}") for L in range(2)]
        # shared scratch (written+consumed inside one DVE block)
        SMN = sb.tile([128, 2 * FD], BF16)
        SGA = sb.tile([128, 3 * FD], BF16)   # [sigma-1] for m,n,h
        UG = sb.tile([128, 3 * FD], BF16)
        P1 = sb.tile([128, FD], BF16)
        P2 = sb.tile([128, FD], BF16)
        P3 = sb.tile([128, FD], BF16)
        ISUM = sb.tile([128, FD], BF16)
        VRST = sb.tile([128, FD], BF16)
        RATE = sb.tile([128, FD], F32)
        OUTS = sb.tile([128, BC], F32)

        BIASC = sb.tile([128, 10], F32)      # activation bias constants

        accp = pp.tile([128, FD], F32)
        i0p = pp.tile([128, FD], F32)
        outp = pp.tile([128, BC], F32)

        # ---- loads (one DMA per tensor; chunk-major into partitions) --
        # layer-0 input path first: the first DVE block waits on IEXT[0]
        nc.sync.dma_start(xtsb[:].rearrange("p (c n) -> p c n", c=KC0),
                          xT_d[:].rearrange("(c p) n -> p c n", p=128))
        nc.sync.dma_start(b0sb[:], b0_d[:])
        nc.sync.dma_start(w0sb[:].rearrange("p (c m) -> p c m", c=KC0),
                          w0_d[:].rearrange("(c p) m -> p c m", p=128))
        nc.sync.dma_start(w1sb[:].rearrange("p (c m) -> p c m", c=NCH),
                          w1_d[:].rearrange("(c p) m -> p c m", p=128))
        nc.sync.dma_start(b1row[:], b1r_d[:])
        nc.sync.dma_start(idsb[:], id_d[:])
        nc.sync.dma_start(wosb[:].rearrange("p (c o) -> p c o", c=NCH),
                          wo_d[:].rearrange("(c p) o -> p c o", p=128))
        nc.sync.dma_start(bosb[:], bo_d[:])

        # ---- init -----------------------------------------------------
        for L in range(2):
            nc.vector.memset(V[L][:], v_rest)
            nc.vector.memset(G[L][:, :FD], 0.05)
            nc.vector.memset(G[L][:, FD:2 * FD], 0.32)
            nc.vector.memset(G[L][:, 2 * FD:], 0.6)
        nc.vector.memset(VRST[:], v_res)
        nc.gpsimd.memset(ones[:], 1.0)
        bias_vals = [-4.0,                                  # E1
                     -5.5,                                  # E2
                     -65.0 / 20.0 + ln(0.07 * DT),          # AH
                     -65.0 / 18.0 + ln(4.0 * DT),          # BM
                     -65.0 / 80.0 + ln(0.125 * DT),         # BN
                     35.0 / 20.0,                           # TH
                     DT / 2.0 - 1.0,                        # TH2
                     0.0 if b1_const is None else b1_const, # IEXT1 bias
                     0.4,                                   # VOFFm: .01*(v+40)
                     0.055]                                 # VOFFn: .001*(v+55)
        for i, bv in enumerate(bias_vals):
            nc.gpsimd.memset(BIASC[:, i:i + 1], bv)
        (bE1, bE2, bAH, bBM, bBN, bTH, bTH2, bB1, bVm, bVn) = (
            BIASC[:, i:i + 1] for i in range(10))

        # i0 = x_shard @ w_exc0 ;  IEXT[0] = (DT/T)*psum + DT*b0 + beta
        for m in range(NCH):
            for c in range(KC0):
                nc.tensor.matmul(
                    i0p[:, m * BC:(m + 1) * BC],
                    w0sb[:, c * H0 + m * 128: c * H0 + (m + 1) * 128],
                    xtsb[:, c * BC:(c + 1) * BC],
                    start=(c == 0), stop=(c == KC0 - 1))
        for m in range(NCH):
            nc.scalar.activation(IEXT[0][:, m * BC:(m + 1) * BC],
                                 i0p[:, m * BC:(m + 1) * BC],
                                 AF.Identity, bias=b0sb[:, m:m + 1],
                                 scale=DT / T)

        # ---- per-step building blocks --------------------------------
        def rates(L):
            """ScalarE: rate tiles for layer L (reads V[L])."""
            v = V[L][:]
            nc.scalar.activation(E12[L][:, :FD], v, AF.Exp, bias=bE1, scale=-0.1)
            nc.scalar.activation(E12[L][:, FD:], v, AF.Exp, bias=bE2, scale=-0.1)
            nc.scalar.activation(VOFF[L][:, :FD], v, AF.Identity, bias=bVm,
                                 scale=0.01)
            nc.scalar.activation(VOFF[L][:, FD:], v, AF.Identity, bias=bVn,
                                 scale=0.001)
            nc.scalar.activation(BMN[L][:, :FD], v, AF.Exp, bias=bBM,
                                 scale=-1.0 / 18.0)
            nc.scalar.activation(BMN[L][:, FD:], v, AF.Exp, bias=bBN,
                                 scale=-1.0 / 80.0)
            nc.scalar.activation(TH[L][:], v, AF.Tanh, bias=bTH, scale=1.0 / 20.0)
            nc.scalar.activation(TH2[L][:], TH[L][:], AF.Identity, bias=bTH2,
                                 scale=DT / 2.0)
            nc.scalar.activation(A[L][:, 2 * FD:], v, AF.Exp, bias=bAH,
                                 scale=-1.0 / 20.0)

        def dve_block(L, iext=None):
            """DVE: full HH update for layer L (gates, currents, v, spike)."""
            v = V[L][:]
            iext = IEXT[L][:] if iext is None else iext
            g = G[L][:]
            stt = nc.vector.scalar_tensor_tensor
            tt = nc.vector.tensor_tensor
            # rational rates in one [1024] custom: A[:2FD] = [DT*a_m | DT*a_n]
            nc.vector._custom_dve(op_rate, out=A[L][:, :2 * FD],
                                  in0=E12[L][:], in1=VOFF[L][:],
                                  s0=_RC0, s1=_RC1, imm2=0.0)
            # sigma-1 for all gates -> SGA [m|n|h]
            tt(SMN[:], A[L][:, :2 * FD], BMN[L][:], ALU.add)
            nc.vector.tensor_scalar(SGA[:, :2 * FD], SMN[:], 1.0, None,
                                    ALU.subtract)
            tt(SGA[:, 2 * FD:], TH2[L][:], A[L][:, 2 * FD:], ALU.add)
            # fused 3-gate update: g' = DT*a - (sigma-1)*g
            tt(UG[:], SGA[:], g, ALU.mult)
            tt(g, A[L][:], UG[:], ALU.subtract)
            # currents: P1 = DT*gna*m^3 h ; P2 = DT*gk*n^4*(v-ek)
            nc.vector._custom_dve(op_m3h, out=P1[:], in0=G[L][:, :FD],
                                  in1=G[L][:, 2 * FD:], s0=DT * gna, s1=0.0,
                                  imm2=0.0)
            nc.vector._custom_dve(op_n4v, out=P2[:], in0=G[L][:, FD:2 * FD],
                                  in1=v, s0=ek, s1=DT * gk, imm2=0.0)
            stt(P3[:], v, ena, P1[:], ALU.subtract, ALU.mult)
            tt(ISUM[:], P3[:], P2[:], ALU.add)
            # v' = (alpha*v - ISUM) + IEXT ; only the last (short) op
            # depends on IEXT, which for layer 1 arrives late via PE+copy
            stt(V1T[L][:], v, alpha, ISUM[:], ALU.mult, ALU.subtract)
            tt(v, V1T[L][:], iext, ALU.add)
            # spike + reset (bf16 spike tile doubles as the int mask)
            nc.vector.tensor_scalar(S[L][:], v, v_th, None, ALU.is_gt)
            nc.vector.copy_predicated(v, S[L][:].bitcast(mybir.dt.uint16),
                                      VRST[:])

        def pe_i1(k):
            """i1 = s0 @ (DT*W1) + (DT*b1 + beta) into fresh PSUM tile."""
            i1p = pi.tile([128, FD], F32, tag="i1p")
            for m in range(NCH):
                for c in range(NCH):
                    nc.tensor.matmul(
                        i1p[:, m * BC:(m + 1) * BC],
                        w1sb[:, c * H1 + m * 128: c * H1 + (m + 1) * 128],
                        S[0][:, c * BC:(c + 1) * BC],
                        start=(c == 0), stop=False)
                nc.tensor.matmul(
                    i1p[:, m * BC:(m + 1) * BC],
                    b1row[0:1, m * 128:(m + 1) * 128],
                    ones[0:1, :],
                    start=False, stop=True)
            return i1p

        # ---- the pipelined timestep loop -----------------------------
        rates(0)
        for k in range(T):
            rates(1)                       # ScalarE (runs during DVE layer-0)
            dve_block(0)                   # DVE layer 0 step k
            if debug == 2 and k == 0:
                srcs = dict(SMN=SMN, P1=P1, P2=P2, P3=P3,
                            ISUM=ISUM, V1T=V1T[0], IEXT=IEXT[0])
                for n, t in srcs.items():
                    fd = t.shape[-1]
                    DBGS = sb.tile([128, fd], F32, name=f"DBGS_{n}")
                    nc.vector.tensor_copy(DBGS[:], t[:])
                    nc.sync.dma_start(dbgs_d[n][:, :fd], DBGS[:])
            if k + 1 < T:
                rates(0)                   # ScalarE (runs during DVE layer-1)
            i1p = pe_i1(k)                 # PE
            dve_block(1, iext=i1p[:])      # DVE layer 1 step k (PSUM read)
            nc.tensor.matmul(accp[:], idsb[:], S[1][:],
                             start=(k == 0), stop=(k == T - 1),
                             skip_group_check=True)

        # ---- readout: (acc/T) @ w_out + b_out ------------------------
        nc.scalar.activation(RATE[:], accp[:], AF.Identity, bias=0.0, scale=1.0 / T)
        for c in range(NCH):
            nc.tensor.matmul(outp[:],
                             wosb[:, c * OUT:(c + 1) * OUT],
                             RATE[:, c * BC:(c + 1) * BC],
                             start=(c == 0), stop=(c == NCH - 1))
        nc.scalar.activation(OUTS[:], outp[:], AF.Identity, bias=bosb[:, 0:1], scale=1.0)
        nc.sync.dma_start(out_d[:], OUTS[:])

        if debug:
            dbg_src = [V[0][:], G[0][:, :FD], G[0][:, 2 * FD:], G[0][:, FD:2 * FD],
                       V[1][:], G[1][:, :FD], G[1][:, 2 * FD:], G[1][:, FD:2 * FD]]
            for i, sap in enumerate(dbg_src):
                DBG = sb.tile([128, FD], F32, name=f"DBG{i}")
                nc.vector.tensor_copy(DBG[:], sap)
                nc.sync.dma_start(dbg_d[i][:], DBG[:])
            nc.scalar.activation(RATE[:], accp[:], AF.Identity, bias=0.0, scale=1.0)
            nc.sync.dma_start(dbga_d[:], RATE[:])
    nc.compile()
    return nc


_NC_CACHE = {}


def _get_nc(T, scal, debug=False, b1_const=None):
    key = (T, tuple(sorted(scal.items())), debug, b1_const)
    if key not in _NC_CACHE:
        _NC_CACHE[key] = _build(T, scal, debug, b1_const=b1_const)
    return _NC_CACHE[key]


def _chunk_major(vec):
    """[1024] -> [128, 8] with vec[c*128+p] at [p, c]."""
    return np.ascontiguousarray(vec.reshape(NCH, 128).T)


def _make_in_maps(inputs, T, scal):
    gl = scal["g_leak"]; v_rest = scal["v_rest"]
    beta = DT * gl * v_rest

    x = np.asarray(inputs["x"], np.float32)
    w_exc0 = np.ascontiguousarray(np.asarray(inputs["w_exc0"], np.float32))
    W1 = np.concatenate([np.asarray(inputs["w_exc1"], np.float32),
                         -np.asarray(inputs["w_inh1"], np.float32)], axis=0)
    w1dt = (DT * W1).astype(ml_dtypes.bfloat16)
    b0dt = (_chunk_major(DT * np.asarray(inputs["b_exc0"], np.float32)) + beta
            ).astype(np.float32)
    b1row = (DT * (np.asarray(inputs["b_exc1"], np.float32)
                   - np.asarray(inputs["b_inh1"], np.float32)) + beta
             ).reshape(1, H1).astype(ml_dtypes.bfloat16)
    w_out = np.ascontiguousarray(np.asarray(inputs["w_out"], np.float32))
    b_out = np.asarray(inputs["b_out"], np.float32).reshape(128, 1)
    ident = np.eye(128, dtype=ml_dtypes.bfloat16)

    in_maps = []
    for c in range(NCORES):
        xT = np.ascontiguousarray(x[c * BC:(c + 1) * BC, :].T)
        in_maps.append({
            "xT": xT, "w_exc0": w_exc0, "b0dt": b0dt, "w1dt": w1dt,
            "b1row": b1row, "w_out": w_out, "b_out": b_out, "ident": ident,
        })
    return in_maps


def _b1_const(inputs, scal):
    """If b_exc1 - b_inh1 is a uniform constant, the PE bias rows can be
    dropped and DT*b1 + beta folded into the IEXT copy bias."""
    b1 = (np.asarray(inputs["b_exc1"], np.float32)
          - np.asarray(inputs["b_inh1"], np.float32))
    if np.all(b1 == b1.flat[0]):
        beta = DT * scal["g_leak"] * scal["v_rest"]
        return float(DT * b1.flat[0] + beta)
    return None


def kernel(**inputs):
    T = int(np.asarray(inputs["timesteps"]))
    scal = {k: float(np.asarray(inputs[k])) for k in
            ("v_rest", "v_threshold", "v_reset", "g_na_max", "g_k_max",
             "g_leak", "e_na", "e_k")}
    nc = _get_nc(T, scal, b1_const=_b1_const(inputs, scal))
    in_maps = _make_in_maps(inputs, T, scal)
    res = run_bass_kernel_spmd(nc, in_maps, core_ids=list(range(NCORES)))
    out = np.empty((B, OUT), np.float32)
    for c in range(NCORES):
        out[c * BC:(c + 1) * BC, :] = res.results[c]["out"].T
    return out
